# revision 31
# baseline (speedup 1.0000x reference)
"""GRU layer (flax GRUCell math) on 8 Trainium2 NeuronCores.

Data-parallel: batch 64 sharded 8-way (8 rows/core); weights replicated;
the T=4096 recurrence runs locally per core.

Layout strategy (v2): x is pre-transposed on the HOST to [D, T, B_loc] per
core and y is produced as [H, T, B_loc] then post-transposed on the host.
This makes each chunk's input/output a single contiguous [128, C*B] DMA and
removes all on-device transposes.

Per-core compute, per chunk of C=64 steps (BT = C*B = 512 columns):
  - PSUM holds pre-activation accumulators: prz [128, 2*BT] (r|z gates) and
    pnh [128, BT] (n-gate h-side + b_hn), initialized per chunk with
    bias-broadcast (K=1) matmuls + the x-side GEMMs; per-step h-side
    contributions accumulate on top (start=False).
  - gn = x@Win + b_in staged to SBUF via two activations per chunk.
  - scan step t, critical chain: sigmoid on [r|z] straight from PSUM (ACT)
    -> v = r*pnh (DVE) -> w = v+gn (DVE) -> n = tanh(w) (ACT) ->
    q = n*(1-z) (DVE) -> 3 q-matmuls (PE) -> next step's sigmoid.
    Off-chain DVE work is made w-DEPENDENT via fused scalar_tensor_tensor
    forms so the greedy scheduler cannot hoist it into the chain; it fills
    the tanh window: omz = (w<=1e30) - z = 1-z (exact), t1 = w*0 + z = z,
    u = t1*h_prev, h = q+u. The h-side matmuls for step t+1 use the linear
    split h@W = u@W + q@W so the blend add stays off the critical path
    (u-matmuls run during tanh, q-matmuls right after q).
  - x-side operands are float32r: same fp32 bytes, but the PE streams them
    at 1 cycle/row instead of fp32's 4 (~2e-4 rel err, PSUM accum fp32).
  - chunk prep for chunk c+2 is emitted dripped into the scan of chunk c so
    its GEMMs fill PE idle slots instead of stalling the chunk boundary.
"""

import sys

sys.path.insert(0, "/opt/trn_rl_repo")

import numpy as np

import concourse.bacc as bacc
import concourse.tile as tile
from concourse import mybir
from concourse.bass_utils import run_bass_kernel_spmd

F32 = mybir.dt.float32
F32R = mybir.dt.float32r
AF = mybir.ActivationFunctionType
OP = mybir.AluOpType

B, T, D, H = 64, 4096, 128, 128
NCORES = 8
BL = B // NCORES  # 8 batch rows per core


def build_gru_nc(BL=BL, T=T, C=64):
    """Build the single-core GRU program (SPMD-replicated across cores)."""
    assert T % C == 0
    BT = C * BL  # columns per chunk in [*, (t, b)] layout
    assert BT * 4 == 2048, "each gate region must be exactly one PSUM bank"
    NCH = T // C

    nc = bacc.Bacc("TRN2", target_bir_lowering=False, debug=False)

    # x pre-transposed on host: [d][t][b]. The x-side operands are float32r
    # (same 4-byte layout as fp32) so the big prep GEMMs stream at
    # 1 cycle/row instead of fp32's 4.
    x_d = nc.dram_tensor("xT", [D, T, BL], F32R, kind="ExternalInput").ap()
    wi_d = nc.dram_tensor("wi", [D, 3 * H], F32R, kind="ExternalInput").ap()
    wh_d = nc.dram_tensor("wh", [H, 3 * H], F32, kind="ExternalInput").ap()
    # b_row = [b_ir | b_iz | b_hn] as a row vector for K=1 broadcast matmuls
    brow_d = nc.dram_tensor("b_row", [1, 3 * H], F32R, kind="ExternalInput").ap()
    bin_d = nc.dram_tensor("b_in", [H, 1], F32, kind="ExternalInput").ap()
    ones_d = nc.dram_tensor("ones_row", [1, BT], F32R, kind="ExternalInput").ap()
    # y transposed: [h][t][b]; host transposes back
    y_d = nc.dram_tensor("y", [H, T, BL], F32, kind="ExternalOutput").ap()

    with tile.TileContext(nc) as tc:
        with (
            tc.tile_pool(name="const", bufs=1) as const_p,
            tc.tile_pool(name="xt", bufs=2) as xt_p,
            tc.tile_pool(name="gn", bufs=2) as gn_p,
            tc.tile_pool(name="hs", bufs=2) as hs_p,
            tc.tile_pool(name="small", bufs=4) as small_p,
            tc.tile_pool(name="prz", bufs=2, space="PSUM") as prz_p,
            tc.tile_pool(name="pnh", bufs=2, space="PSUM") as pnh_p,
            tc.tile_pool(name="pgn", bufs=1, space="PSUM") as pgn_p,
        ):
            wi = const_p.tile([D, 3 * H], F32R)
            nc.sync.dma_start(wi[:], wi_d)
            wh = const_p.tile([H, 3 * H], F32)
            nc.sync.dma_start(wh[:], wh_d)
            brow = const_p.tile([1, 3 * H], F32R)
            nc.sync.dma_start(brow[:], brow_d)
            bin_ = const_p.tile([H, 1], F32)
            nc.sync.dma_start(bin_[:], bin_d)
            ones = const_p.tile([1, BT], F32R)
            nc.sync.dma_start(ones[:], ones_d)

            def prep_steps(c):
                """Chunk-c prep as a list of emission thunks, dripped into the
                running scan so each float32r GEMM (~213ns) lands in a PE idle
                window instead of stalling the chunk boundary."""
                t0 = c * C
                xt = xt_p.tile([D, BT], F32R, tag="xt", name=f"xt{c}")
                prz = prz_p.tile([128, 2 * BT], F32, tag="prz", name=f"prz{c}")
                pnh = pnh_p.tile([128, BT], F32, tag="pnh", name=f"pnh{c}")
                pgn = pgn_p.tile([128, BT], F32, tag="pgn", name=f"pgn{c}")
                gn = gn_p.tile([128, BT], F32, tag="gn", name=f"gn{c}")
                chunks[c] = (prz, pnh, gn)
                return [
                    lambda: nc.sync.dma_start(xt[:], x_d[:, t0 : t0 + C, :]),
                    lambda: nc.tensor.matmul(prz[:, 0:BT], brow[:, 0:H], ones[:], start=True, stop=False),
                    lambda: nc.tensor.matmul(prz[:, BT : 2 * BT], brow[:, H : 2 * H], ones[:], start=True, stop=False),
                    lambda: nc.tensor.matmul(pnh[:], brow[:, 2 * H : 3 * H], ones[:], start=True, stop=False),
                    lambda: nc.tensor.matmul(prz[:, 0:BT], wi[:, 0:H], xt[:], start=False, stop=False),
                    lambda: nc.tensor.matmul(prz[:, BT : 2 * BT], wi[:, H : 2 * H], xt[:], start=False, stop=False),
                    lambda: nc.tensor.matmul(pgn[:], wi[:, 2 * H : 3 * H], xt[:], start=True, stop=True),
                    lambda: nc.scalar.activation(gn[:, 0 : BT // 2], pgn[:, 0 : BT // 2], AF.Identity, bias=bin_[:]),
                    lambda: nc.scalar.activation(gn[:, BT // 2 : BT], pgn[:, BT // 2 : BT], AF.Identity, bias=bin_[:]),
                ]

            chunks = {}
            for thunk in prep_steps(0):
                thunk()
            if NCH > 1:
                for thunk in prep_steps(1):
                    thunk()
            prev_stage = None
            for c in range(NCH):
                prz, pnh, gn = chunks[c]
                prz3 = prz.rearrange("p (g c) -> p g c", g=2)
                stage = hs_p.tile([H, BT], F32, tag="hs", name=f"hs{c}")
                pending = prep_steps(c + 2) if c + 2 < NCH else []
                for tl in range(C):
                    # drip one prep emission every 4 steps starting at tl=8
                    if pending and tl >= 8 and tl % 4 == 0:
                        pending.pop(0)()
                    cs = slice(tl * BL, (tl + 1) * BL)
                    # --- critical chain: sigmoid -> v -> w -> tanh -> q ---
                    rz = small_p.tile([H, 2 * BL], F32, tag="rz")
                    nc.scalar.activation(
                        rz[:].rearrange("p (g c) -> p g c", g=2),
                        prz3[:, :, cs],
                        AF.Sigmoid,
                    )
                    v = small_p.tile([H, BL], F32, tag="v")
                    nc.vector.tensor_mul(v[:], pnh[:, cs], rz[:, 0:BL])
                    w = small_p.tile([H, BL], F32, tag="w")
                    nc.vector.tensor_add(w[:], v[:], gn[:, cs])
                    # off-chain DVE work is made w-DEPENDENT via fused
                    # scalar_tensor_tensor forms so the greedy scheduler cannot
                    # hoist it between v and w (the ops run in the tanh window,
                    # and keeping everything off Pool leaves every engine's
                    # wait single-condition):
                    #   omz = (w <= 1e30) - z = 1 - z        (exact)
                    #   t1  = (w * 0) + z     = z            (w-gated copy)
                    omz = small_p.tile([H, BL], F32, tag="omz")
                    if c == 0 and tl == 0:
                        h_prev = None
                    elif tl == 0:
                        h_prev = prev_stage[:, (C - 1) * BL : C * BL]
                    else:
                        h_prev = stage[:, (tl - 1) * BL : tl * BL]
                    u = None
                    if h_prev is not None:
                        t1 = small_p.tile([H, BL], F32, tag="t1")
                        nc.vector.scalar_tensor_tensor(
                            t1[:], w[:], 0.0, rz[:, BL : 2 * BL], OP.mult, OP.add
                        )
                        u = small_p.tile([H, BL], F32, tag="u")
                        nc.vector.tensor_mul(u[:], t1[:], h_prev)
                    # omz depends on u (not just w) so the scheduler runs u
                    # first: the u-matmuls then fire earlier and the q-matmuls'
                    # wait registers sooner (cheap +31 PE entry):
                    #   omz = (u <= 1e30) - t1 = 1 - z   (exact)
                    nc.vector.scalar_tensor_tensor(
                        omz[:], u[:] if u is not None else w[:], 1e30, t1[:] if u is not None else rz[:, BL : 2 * BL], OP.is_le, OP.subtract
                    )
                    # targets for the h-side accumulation of step t+1
                    if tl < C - 1:
                        ns = slice((tl + 1) * BL, (tl + 2) * BL)
                        tprz, tpnh = prz, pnh
                    elif c + 1 < NCH:
                        ns = slice(0, BL)
                        tprz, tpnh = chunks[c + 1][0], chunks[c + 1][1]
                    else:
                        tprz = None
                    pass
                    # --- chain tail: tanh -> q -> q-matmuls ---
                    n = small_p.tile([H, BL], F32, tag="n")
                    nc.scalar.activation(n[:], w[:], AF.Tanh)
                    q = small_p.tile([H, BL], F32, tag="q")
                    nc.vector.tensor_mul(q[:], n[:], omz[:])
                    if tprz is not None:
                        # interleave u- and q-matmuls per gate: each q-matmul's
                        # wait registers right after the (early) u-matmul ahead
                        # of it completes, crossing the PE prepay threshold.
                        zs = slice(BT + ns.start, BT + ns.stop)
                        if u is not None:
                            nc.tensor.matmul(tprz[:, ns], wh[:, 0:H], u[:], start=False, stop=False)
                        nc.tensor.matmul(tprz[:, ns], wh[:, 0:H], q[:], start=False, stop=True)
                        if u is not None:
                            nc.tensor.matmul(tprz[:, zs], wh[:, H : 2 * H], u[:], start=False, stop=False)
                        nc.tensor.matmul(tprz[:, zs], wh[:, H : 2 * H], q[:], start=False, stop=True)
                        if u is not None:
                            nc.tensor.matmul(tpnh[:, ns], wh[:, 2 * H : 3 * H], u[:], start=False, stop=False)
                        nc.tensor.matmul(tpnh[:, ns], wh[:, 2 * H : 3 * H], q[:], start=False, stop=True)
                    # h = q + u (off-chain; feeds next step's u and the output DMA)
                    if u is not None:
                        nc.vector.tensor_add(stage[:, cs], q[:], u[:])
                    else:
                        nc.vector.tensor_copy(stage[:, cs], q[:])
                nc.sync.dma_start(y_d[:, c * C : (c + 1) * C, :], stage[:])
                prev_stage = stage

    nc.compile()
    return nc


_NC_CACHE = {}


def _get_nc(BL_, T_, C_):
    key = (BL_, T_, C_)
    if key not in _NC_CACHE:
        _NC_CACHE[key] = build_gru_nc(BL_, T_, C_)
    return _NC_CACHE[key]


def make_in_maps(x, Wir, Wiz, Win, Whr, Whz, Whn, b_ir, b_iz, b_in, b_hn):
    """Host-side prep: weight concat + per-core x transpose to [D, T, BL]."""
    x = np.asarray(x, dtype=np.float32)
    Bx = x.shape[0]
    bl = Bx // NCORES
    wi = np.ascontiguousarray(np.concatenate([Wir, Wiz, Win], axis=1).astype(np.float32))
    wh = np.ascontiguousarray(np.concatenate([Whr, Whz, Whn], axis=1).astype(np.float32))
    brow = np.ascontiguousarray(
        np.concatenate([b_ir, b_iz, b_hn])[None, :].astype(np.float32)
    )
    bin_ = np.ascontiguousarray(np.asarray(b_in, dtype=np.float32)[:, None])
    ones_row = np.ones((1, 512), np.float32)
    in_maps = []
    for i in range(NCORES):
        xT = np.ascontiguousarray(x[i * bl : (i + 1) * bl].transpose(2, 1, 0))
        in_maps.append(
            {"xT": xT, "wi": wi, "wh": wh, "b_row": brow, "b_in": bin_, "ones_row": ones_row}
        )
    return in_maps, bl


def run_gru(x, Wir, Wiz, Win, Whr, Whz, Whn, b_ir, b_iz, b_in, b_hn, C=64, trace=False):
    """x: [B, T, D] float32 (B divisible by NCORES). Returns [B, T, H], plus results obj."""
    Bx, Tx, Dx = np.asarray(x).shape
    in_maps, bl = make_in_maps(
        x, Wir, Wiz, Win, Whr, Whz, Whn, b_ir, b_iz, b_in, b_hn
    )
    nc = _get_nc(bl, Tx, C)
    res = run_bass_kernel_spmd(nc, in_maps, list(range(NCORES)), trace=trace)
    # results[i]["y"] is [H, T, bl]; transpose back to [bl, T, H]
    y = np.concatenate(
        [res.results[i]["y"].transpose(2, 1, 0) for i in range(NCORES)], axis=0
    )
    return np.ascontiguousarray(y), res


def kernel(**inputs) -> np.ndarray:
    inputs = {k: np.asarray(v) for k, v in inputs.items()}
    y, _ = run_gru(**inputs)
    return y.astype(np.float32)


if __name__ == "__main__":
    # smoke test with tiny T against a local numpy GRU reference
    rng = np.random.default_rng(0)
    Ts = 128
    s_i, s_h = 1.0 / np.sqrt(D), 1.0 / np.sqrt(H)
    inp = {
        "x": rng.standard_normal((B, Ts, D), dtype=np.float32),
        "Wir": rng.uniform(-s_i, s_i, (D, H)).astype(np.float32),
        "Wiz": rng.uniform(-s_i, s_i, (D, H)).astype(np.float32),
        "Win": rng.uniform(-s_i, s_i, (D, H)).astype(np.float32),
        "Whr": rng.uniform(-s_h, s_h, (H, H)).astype(np.float32),
        "Whz": rng.uniform(-s_h, s_h, (H, H)).astype(np.float32),
        "Whn": rng.uniform(-s_h, s_h, (H, H)).astype(np.float32),
        "b_ir": rng.uniform(-s_i, s_i, (H,)).astype(np.float32),
        "b_iz": rng.uniform(-s_i, s_i, (H,)).astype(np.float32),
        "b_in": rng.uniform(-s_i, s_i, (H,)).astype(np.float32),
        "b_hn": rng.uniform(-s_h, s_h, (H,)).astype(np.float32),
    }

    def np_gru(x, Wir, Wiz, Win, Whr, Whz, Whn, b_ir, b_iz, b_in, b_hn):
        Bx, Tx, _ = x.shape
        h = np.zeros((Bx, H), np.float32)
        gi_r = x @ Wir + b_ir
        gi_z = x @ Wiz + b_iz
        gi_n = x @ Win + b_in
        out = np.zeros((Bx, Tx, H), np.float32)
        for t in range(Tx):
            r = 1 / (1 + np.exp(-(gi_r[:, t] + h @ Whr)))
            z = 1 / (1 + np.exp(-(gi_z[:, t] + h @ Whz)))
            n = np.tanh(gi_n[:, t] + r * (h @ Whn + b_hn))
            h = (1 - z) * n + z * h
            out[:, t] = h
        return out

    expected = np_gru(**inp)
    y, _ = run_gru(**inp, C=64)
    err = np.abs(y - expected).max() / (np.abs(expected).max() + 1e-30)
    print("max abs err (rel to absmax):", err)
    assert err < 2e-3, err
    print("SMOKE TEST PASSED")
